# revision 17
# baseline (speedup 1.0000x reference)
"""BoundaryAttentionModule Trainium2 kernel.

Shapes (hardcoded): b=4, c=256, h=w=64 (HW=4096), boundary 128x128,
mid=64, out_ch=256. 8 cores: core = (batch bi = core//2, key-half kh = core%2).

Math (exact reassociation of the reference):
  bm   = nearest-downsampled boundary map        [b, 4096]
  R    = relu(kw1f outer bm_khalf + beta)        [64, 2048]   (kw1f = key_w1*bn_inv)
  G    = (key_w2^T @ query_w) @ u                [64, 4096]
  E^T  = R^T @ G                                 [2048_k, 4096_j]  (logits tiny, no max sub)
  U    = exp(E^T), s[k] = sum_j U[k, j]
  Vt   = (u^T @ value_w^T)[k_half] / s * 8192    [2048, 256]
  P    = Vt^T @ U                                [256, 4096]  per-core partial (x8192)
host: out[bi] = (gamma/8192) * (P[2bi] + P[2bi+1]) + u[bi]

Schedule: the ACT engine (exp, 64 Melem/core at 1.2GHz) is the bottleneck;
everything else hides under its shadow.  Energy matmuls are bf16 K=64
partition-half duos into 2x[128,1024] rotating PSUM buffers; exp streams
chunk-by-chunk into fp8 U tiles.  Row-sums ride Pool (tensor_scalar
accum_out, first j-half) and DVE (reduce, second j-half).  The output
matmul (fp8 DoubleRow, 2 keys/cell) is split: j in [0,1024) accumulates
per-pair into two pinned PSUM banks *during* the exp loop (emitted one
k-tile late so the PE pipeline never stalls ACT); j in [1024,4096) runs
as a short phase C afterwards.  Vt matmuls borrow the pinned banks early
in the loop, with stationary operands read straight from u via stride-2
access patterns (key order is the pairwise DoubleRow interleave).
Per-core j columns are host-rolled so each core's own key half occupies
columns [0,2048); the host un-rolls the bf16 partial outputs.
"""

import numpy as np

B, C, HW = 4, 256, 4096
KH = HW // 2          # 2048 keys per core
NK = KH // 128        # 16 k tiles
NP = NK // 2          # 8 k-tile pairs
MID = 64
JC = 1024             # j chunk width (energy/exp)
NCH = HW // JC        # 4 chunks per k-tile
VSCALE = 8192.0       # fp8 scaling of Vt (power of two; host divides gamma)

TRACE = False
TRACE_CORES = None
LAST_RESULTS = None

_BUILT = None


def _build():
    import concourse.bass as bass
    import concourse.tile as tile
    from concourse import bacc, mybir

    f32 = mybir.dt.float32
    bf16 = mybir.dt.bfloat16
    fp8 = mybir.dt.float8e4
    AF = mybir.ActivationFunctionType
    AX = mybir.AxisListType
    ALU = mybir.AluOpType
    DR = mybir.MatmulPerfMode.DoubleRow

    nc = bacc.Bacc(
        "TRN2",
        target_bir_lowering=False,
        debug=False,
        enable_asserts=False,
        num_devices=8,
    )

    u_in = nc.dram_tensor("u_in", [C, HW], bf16, kind="ExternalInput").ap()
    # wp_in packs [mt | mt1 | vwt0 | vwt1] column-wise; kb_in packs [kw1f2 | bmk]
    wp_in = nc.dram_tensor("wp_in", [128, 768], bf16, kind="ExternalInput").ap()
    kb_in = nc.dram_tensor("kb_in", [1, 2 * MID + KH], bf16,
                           kind="ExternalInput").ap()
    beta_in = nc.dram_tensor("beta_in", [2 * MID, 1], f32, kind="ExternalInput").ap()
    out_d = nc.dram_tensor("outp", [C, HW], bf16, kind="ExternalOutput").ap()

    with tile.TileContext(nc) as tc:
        with (
            tc.tile_pool(name="sb", bufs=1) as sb,
            tc.tile_pool(name="ost", bufs=3) as osp,
            tc.tile_pool(name="ps", bufs=2, space="PSUM") as ps,
        ):
            # ---- inputs; packed weights on the idle ACT queue, u halves on
            # sync/gpsimd with staggered emission for fine-grained sem waits ----
            wp = sb.tile([128, 768], bf16, tag="wp", name="wp")
            nc.scalar.dma_start(wp[:, :], wp_in[:, :])
            kb = sb.tile([1, 2 * MID + KH], bf16, tag="kb", name="kb")
            nc.scalar.dma_start(kb[:, :], kb_in[:, :])
            betat = sb.tile([2 * MID, 1], f32, tag="betat", name="betat")
            nc.scalar.dma_start(betat[:], beta_in[:, :])
            mt, mt1 = wp[:, 0:128], wp[:, 128:256]
            vwt0, vwt1 = wp[:, 256:512], wp[:, 512:768]
            kw1, bmk = kb[:, 0 : 2 * MID], kb[:, 2 * MID :]
            u0 = sb.tile([128, HW], bf16, tag="u0", name="u0")
            u1 = sb.tile([128, HW], bf16, tag="u1", name="u1")
            for jo in range(0, 2 * JC, JC):
                nc.sync.dma_start(u0[:, jo : jo + JC], u_in[0:128, jo : jo + JC])
                nc.gpsimd.dma_start(u1[:, jo : jo + JC], u_in[128:256, jo : jo + JC])

            # ---- persistent SBUF state ----
            R2 = sb.tile([128, KH], bf16, tag="R2", name="R2")
            G2 = sb.tile([128, HW], bf16, tag="G2", name="G2")
            u_pairs = []
            for pair in range(NP):
                Up = sb.tile([128, 2 * HW], fp8, tag=f"Up{pair}", name=f"Up{pair}")
                u_pairs.append(Up)
            vtb = sb.tile([128, NK * C], bf16, tag="vtb", name="vtb")
            vtsp = []
            for pair in range(NP):
                vp = sb.tile([128, 2 * C], fp8, tag=f"vtsp{pair}", name=f"vtsp{pair}")
                vtsp.append(vp)
            s_v = sb.tile([128, NK], f32, tag="s_v", name="s_v")
            rinv = sb.tile([128, NK], f32, tag="rinv", name="rinv")
            sp_tail = {}
            for kt in (NK - 2, NK - 1):
                sp_tail[kt] = sb.tile([128, NCH], f32, tag=f"sp{kt}",
                                      name=f"sp{kt}")

            # PSUM: 2 rotating energy buffers + 2 pinned output accumulators
            pp0 = ps.tile([128, JC], f32, tag="pp0", bufs=1, name="pp0")
            pp1 = ps.tile([128, JC], f32, tag="pp1", bufs=1, name="pp1")
            pps = [pp0, pp1]

            # stride-2 view of u: col = blk*256 + q*2 + two  (keys of pair blk)
            u0v = u0.rearrange("p (blk q two) -> p blk two q", two=2, q=128)
            u1v = u1.rearrange("p (blk q two) -> p blk two q", two=2, q=128)

            # ---- R2 = relu(kw1f2 outer bmk + beta2)  [128, 2048] ----
            for rc in range(2):
                pr = ps.tile([128, JC], f32, tag="pe", name=f"pr{rc}")
                for q in range(2):
                    nc.tensor.matmul(
                        pr[:, q * 512 : (q + 1) * 512],
                        kw1[:, :],
                        bmk[:, rc * 1024 + q * 512 : rc * 1024 + (q + 1) * 512],
                        start=True, stop=True,
                    )
                nc.vector.tensor_scalar(
                    R2[:, rc * 1024 : (rc + 1) * 1024], pr[:, :],
                    betat[:, 0:1], 0.0, op0=ALU.add, op1=ALU.max,
                )

            # tail u chunks, emitted late so early consumers don't wait on them
            for jo in range(2 * JC, HW, JC):
                nc.sync.dma_start(u0[:, jo : jo + JC], u_in[0:128, jo : jo + JC])
                nc.gpsimd.dma_start(u1[:, jo : jo + JC], u_in[128:256, jo : jo + JC])

            # ---- G2 = M2 @ u (both halves)  [128, 4096] ----
            for ci in range(NCH):
                pg = ps.tile([128, JC], f32, tag="pe", name=f"pg{ci}")
                for q in range(2):
                    js = ci * JC + q * 512
                    sl = slice(q * 512, (q + 1) * 512)
                    nc.tensor.matmul(
                        pg[:, sl], mt[:, :], u0[:, js : js + 512],
                        start=True, stop=False,
                    )
                    nc.tensor.matmul(
                        pg[:, sl], mt1[:, :], u1[:, js : js + 512],
                        start=False, stop=True,
                    )
                nc.vector.tensor_copy(G2[:, ci * JC : (ci + 1) * JC], pg[:, :])

            def vt_round(r):
                """Vt for k-tiles 4r..4r+3 into the (not yet pinned) pp banks."""
                pv = pps[r % 2]
                for t in range(4):
                    kt = 4 * r + t
                    pair, hf = kt // 2, kt % 2
                    nc.tensor.matmul(
                        pv[:, t * 256 : (t + 1) * 256],
                        u0v[:, pair, hf, :], vwt0[:, :],
                        start=True, stop=False,
                    )
                    nc.tensor.matmul(
                        pv[:, t * 256 : (t + 1) * 256],
                        u1v[:, pair, hf, :], vwt1[:, :],
                        start=False, stop=True,
                    )
                nc.vector.tensor_copy(
                    vtb[:, 4 * r * 256 : (4 * r + 4) * 256], pv[:, :]
                )

            def p_stream(pair):
                """Output-matmul contribution of `pair` for j in [0, JC)."""
                lhs = vtsp[pair].rearrange("p (i c) -> p i c", i=2)
                rhs = u_pairs[pair].rearrange("p (i j) -> p i j", i=2)
                for ct in range(2):
                    for q in range(2):
                        nc.tensor.matmul(
                            pps[ct][:, q * 512 : (q + 1) * 512],
                            lhs[:, :, ct * 128 : (ct + 1) * 128],
                            rhs[:, :, q * 512 : (q + 1) * 512],
                            start=(pair == 0), stop=(pair == NP - 1),
                            perf_mode=DR,
                        )

            # ---- main loop: energy + exp + row-sums + scales + streamed P ----
            for kt in range(NK):
                pair, hf = kt // 2, kt % 2
                Up = u_pairs[pair]
                ride = kt in sp_tail  # last pair: row-sum rides the ACT accum
                for ci in range(NCH):
                    pe = ps.tile([128, JC], f32, tag="pe", name=f"pe{kt}_{ci}")
                    js = ci * JC
                    nc.tensor.matmul(
                        pe[:, 0:512],
                        R2[0:64, kt * 128 : (kt + 1) * 128],
                        G2[0:64, js : js + 512],
                        start=True, stop=True,
                    )
                    nc.tensor.matmul(
                        pe[:, 512:1024],
                        R2[64:128, kt * 128 : (kt + 1) * 128],
                        G2[64:128, js + 512 : js + 1024],
                        start=True, stop=True,
                    )
                    nc.scalar.activation(
                        Up[:, hf * HW + js : hf * HW + js + JC], pe[:, :], AF.Exp,
                        accum_out=(sp_tail[kt][:, ci : ci + 1] if ride else None),
                    )
                if ride:
                    nc.vector.reduce_sum(
                        s_v[:, kt : kt + 1], sp_tail[kt][:, :], axis=AX.X,
                    )
                else:
                    nc.vector.reduce_sum(
                        s_v[:, kt : kt + 1],
                        Up[:, hf * HW : hf * HW + HW], axis=AX.X,
                    )
                if kt == 0:
                    vt_round(0)
                    vt_round(1)
                elif kt == 1:
                    vt_round(2)
                    vt_round(3)
                if hf == 1:
                    nc.vector.reciprocal(
                        rinv[:, kt - 1 : kt + 1], s_v[:, kt - 1 : kt + 1]
                    )
                    for h2 in (0, 1):
                        nc.gpsimd.tensor_scalar(
                            vtsp[pair][:, h2 * C : (h2 + 1) * C],
                            vtb[:, (kt - 1 + h2) * C : (kt + h2) * C],
                            rinv[:, kt - 1 + h2 : kt + h2], VSCALE,
                            op0=ALU.mult, op1=ALU.mult,
                        )
                # streamed P, two tiles of cushion so the PE never stalls ACT
                if kt >= 3 and kt % 2 == 1:
                    p_stream((kt - 3) // 2)
            p_stream(NP - 2)

            # ---- phase C: P for j in [1024, 4096) + streamed j[0:1024) drain.
            # Groups (1,0)/(1,1) pre-stream pairs 0..6 into the freed energy
            # slots while the final exp/sum/scale chain completes; only their
            # pair-7 matmuls (and the last p_stream) gate on vtsp[7].
            def pc_matmuls(po, jc, ct, pairs):
                for pair in pairs:
                    lhs = vtsp[pair].rearrange("p (i c) -> p i c", i=2)[
                        :, :, ct * 128 : (ct + 1) * 128
                    ]
                    rhs = u_pairs[pair].rearrange("p (i j) -> p i j", i=2)
                    for q in range(2):
                        js = jc * JC + q * 512
                        nc.tensor.matmul(
                            po[:, q * 512 : (q + 1) * 512],
                            lhs, rhs[:, :, js : js + 512],
                            start=(pair == 0), stop=(pair == NP - 1),
                            perf_mode=DR,
                        )

            def pc_writeout(po, jc, ct):
                ost = osp.tile([128, JC], bf16, tag="ost", name=f"ost{jc}_{ct}")
                nc.scalar.copy(ost[:, :], po[:, :])
                q = nc.scalar if ct == 0 else nc.sync
                q.dma_start(
                    out_d[ct * 128 : (ct + 1) * 128, jc * JC : (jc + 1) * JC],
                    ost[:, :],
                )

            po10 = ps.tile([128, JC], f32, tag="pe", name="po10")
            po11 = ps.tile([128, JC], f32, tag="pe", name="po11")
            pc_matmuls(po10, 1, 0, range(NP - 1))
            pc_matmuls(po11, 1, 1, range(NP - 1))
            p_stream(NP - 1)
            pc_matmuls(po10, 1, 0, [NP - 1])
            pc_matmuls(po11, 1, 1, [NP - 1])
            for ct in range(2):
                ost = osp.tile([128, JC], bf16, tag="ost", name=f"osts{ct}")
                nc.scalar.copy(ost[:, :], pps[ct][:, :])
                q = nc.scalar if ct == 0 else nc.sync
                q.dma_start(out_d[ct * 128 : (ct + 1) * 128, 0:JC], ost[:, :])
            pc_writeout(po10, 1, 0)
            pc_writeout(po11, 1, 1)
            for jc in range(2, NCH):
                for ct in range(2):
                    po = ps.tile([128, JC], f32, tag="pe", name=f"po{jc}_{ct}")
                    pc_matmuls(po, jc, ct, range(NP))
                    pc_writeout(po, jc, ct)

    nc.compile()
    return nc


def _get_built():
    global _BUILT
    if _BUILT is None:
        _BUILT = _build()
    return _BUILT


def _kperm():
    """Pairwise interleave within 256-key blocks: new index kt*128+q maps to
    old key  (kt//2)*256 + 2q + (kt%2)."""
    perm = np.empty(KH, np.int64)
    for pair in range(NP):
        base = pair * 256
        perm[pair * 256 : pair * 256 + 128] = base + np.arange(0, 256, 2)
        perm[pair * 256 + 128 : pair * 256 + 256] = base + np.arange(1, 256, 2)
    return perm


def _host_prep(boundary_map, uncertainty_map, key_w1, bn_scale, bn_bias,
               bn_mean, bn_var, key_w2, query_w, value_w):
    import ml_dtypes

    bf16 = ml_dtypes.bfloat16
    b, c, h, w = uncertainty_map.shape
    H0 = boundary_map.shape[2]
    idx = (np.arange(h) * H0) // h
    bm = boundary_map[:, 0][:, idx][:, :, idx].reshape(b, h * w).astype(np.float32)

    inv = bn_scale / np.sqrt(bn_var + 1e-5)
    beta = (bn_bias - bn_mean * inv).astype(np.float32)
    kw1f = (key_w1[:, 0] * inv).astype(np.float32)
    m_t = np.ascontiguousarray((key_w2.T @ query_w).T).astype(np.float32)  # [256, 64]
    # duplicate across partition halves for the energy duo-packing
    kw1f2 = np.concatenate([kw1f, kw1f]).reshape(1, 2 * MID).astype(bf16)
    beta2 = np.concatenate([beta, beta]).reshape(2 * MID, 1).astype(np.float32)
    m_t2 = np.concatenate([m_t, m_t], axis=1).astype(bf16)                 # [256, 128]
    vw_t = value_w.T.astype(bf16)                                          # [256, 256]
    # packed weights: [mt | mt1 | vwt0 | vwt1]  [128, 768]
    wp = np.ascontiguousarray(np.concatenate(
        [m_t2[0:128], m_t2[128:256], vw_t[0:128], vw_t[128:256]], axis=1))
    perm = _kperm()

    in_maps = []
    for core in range(8):
        bi, kh = core // 2, core % 2
        u = uncertainty_map[bi].reshape(c, h * w).astype(np.float32)
        # roll j so this core's own key half occupies columns [0, KH)
        u_dev = np.roll(u, -KH * kh, axis=1).astype(bf16)
        bmk = bm[bi, kh * KH : (kh + 1) * KH][perm]
        kb = np.concatenate(
            [kw1f2, bmk.reshape(1, KH).astype(bf16)], axis=1)  # [1, 128+2048]
        in_maps.append({
            "u_in": np.ascontiguousarray(u_dev),
            "wp_in": wp,
            "kb_in": np.ascontiguousarray(kb),
            "beta_in": beta2,
        })
    return in_maps


def kernel(boundary_map, uncertainty_map, key_w1, bn_scale, bn_bias,
           bn_mean, bn_var, key_w2, query_w, value_w, gamma):
    global LAST_RESULTS
    from concourse.bass_utils import run_bass_kernel_spmd

    nc = _get_built()
    in_maps = _host_prep(
        np.asarray(boundary_map), np.asarray(uncertainty_map), np.asarray(key_w1),
        np.asarray(bn_scale), np.asarray(bn_bias), np.asarray(bn_mean),
        np.asarray(bn_var), np.asarray(key_w2), np.asarray(query_w),
        np.asarray(value_w),
    )
    kwargs = {}
    if TRACE:
        kwargs["trace"] = True
        if TRACE_CORES is not None:
            kwargs["trace_cores"] = TRACE_CORES
    res = run_bass_kernel_spmd(nc, in_maps, core_ids=list(range(8)), **kwargs)
    LAST_RESULTS = res

    b, c, h, w = uncertainty_map.shape
    g = np.float32(np.asarray(gamma).reshape(-1)[0] / VSCALE)
    out = np.empty((b, c, h * w), np.float32)
    um = np.asarray(uncertainty_map)
    for bi in range(b):
        P0 = res.results[2 * bi]["outp"].astype(np.float32)
        P1 = res.results[2 * bi + 1]["outp"].astype(np.float32)
        P = P0 + np.roll(P1, KH, axis=1)  # un-roll core kh=1's j columns
        out[bi] = g * P + um[bi].reshape(c, h * w)
    return out.reshape(b, c, h, w)


# revision 19
# speedup vs baseline: 1.0301x; 1.0301x over previous
"""BoundaryAttentionModule Trainium2 kernel.

Shapes (hardcoded): b=4, c=256, h=w=64 (HW=4096), boundary 128x128,
mid=64, out_ch=256. 8 cores: core = (batch bi = core//2, key-half kh = core%2).

Math (exact reassociation of the reference):
  bm   = nearest-downsampled boundary map        [b, 4096]
  R    = relu(kw1f outer bm_khalf + beta)        [64, 2048]   (kw1f = key_w1*bn_inv)
  G    = (key_w2^T @ query_w) @ u                [64, 4096]
  E^T  = R^T @ G                                 [2048_k, 4096_j]  (logits tiny, no max sub)
  U    = exp(E^T), s[k] = sum_j U[k, j]
  Vt   = (u^T @ value_w^T)[k_half] / s * 8192    [2048, 256]
  P    = Vt^T @ U                                [256, 4096]  per-core partial (x8192)
host: out[bi] = (gamma/8192) * (P[2bi] + P[2bi+1]) + u[bi]

Schedule: the ACT engine (exp: 64 Melem/core at 1.2 GHz, plus ~330ns fixed
cost per instruction) is the bottleneck, so everything else hides under its
shadow and the exp chunks are as wide as PSUM allows: two rotating
[128,2048] buffers (all 8 banks), one exp instruction per buffer.  Energy
matmuls are bf16 K=64 partition-half duos (R and G duplicated into both
halves).  Row-sums are full-row DVE reduces of the fp8 U tiles (the last
pair instead rides the ACT accumulator to shorten the tail chain), vtsp
scales on Pool, Vt matmuls slip into the buffer rotation during the first
four tiles with stationary operands read from u via stride-2 access
patterns (key order is the pairwise DoubleRow interleave).  Phase C (the
fp8 DoubleRow output matmul, 2 keys/cell) streams groups through the same
buffers, pre-streaming pairs 0..6 of the first groups while the final
sum/scale chain completes; copies ride the then-idle ACT engine.
Per-core j columns are host-rolled so each core's own key half occupies
columns [0,2048); the host un-rolls the bf16 partial outputs.
"""

import numpy as np

B, C, HW = 4, 256, 4096
KH = HW // 2          # 2048 keys per core
NK = KH // 128        # 16 k tiles
NP = NK // 2          # 8 k-tile pairs
MID = 64
JC = 2048             # j chunk width (energy/exp)
NCH = HW // JC        # 2 chunks per k-tile
VSCALE = 8192.0       # fp8 scaling of Vt (power of two; host divides gamma)

TRACE = False
TRACE_CORES = None
LAST_RESULTS = None

_BUILT = None


def _build():
    import concourse.bass as bass
    import concourse.tile as tile
    from concourse import bacc, mybir

    f32 = mybir.dt.float32
    bf16 = mybir.dt.bfloat16
    fp8 = mybir.dt.float8e4
    AF = mybir.ActivationFunctionType
    AX = mybir.AxisListType
    ALU = mybir.AluOpType
    DR = mybir.MatmulPerfMode.DoubleRow

    nc = bacc.Bacc(
        "TRN2",
        target_bir_lowering=False,
        debug=False,
        enable_asserts=False,
        num_devices=8,
    )

    u_in = nc.dram_tensor("u_in", [C, HW], bf16, kind="ExternalInput").ap()
    # wp_in packs [mt | mt1 | vwt0 | vwt1] column-wise; kb_in packs [kw1f2 | bmk]
    wp_in = nc.dram_tensor("wp_in", [128, 768], bf16, kind="ExternalInput").ap()
    kb_in = nc.dram_tensor("kb_in", [1, 2 * MID + KH], bf16,
                           kind="ExternalInput").ap()
    beta_in = nc.dram_tensor("beta_in", [2 * MID, 1], f32, kind="ExternalInput").ap()
    out_d = nc.dram_tensor("outp", [C, HW], bf16, kind="ExternalOutput").ap()

    with tile.TileContext(nc) as tc:
        with (
            tc.tile_pool(name="sb", bufs=1) as sb,
            tc.tile_pool(name="ost", bufs=3) as osp,
            tc.tile_pool(name="ps", bufs=2, space="PSUM") as ps,
        ):
            # ---- inputs; packed weights on the ACT queue, u halves on
            # sync/gpsimd with staggered emission for fine-grained sem waits ----
            wp = sb.tile([128, 768], bf16, tag="wp", name="wp")
            nc.scalar.dma_start(wp[:, :], wp_in[:, :])
            kb = sb.tile([1, 2 * MID + KH], bf16, tag="kb", name="kb")
            nc.scalar.dma_start(kb[:, :], kb_in[:, :])
            betat = sb.tile([2 * MID, 1], f32, tag="betat", name="betat")
            nc.scalar.dma_start(betat[:], beta_in[:, :])
            mt, mt1 = wp[:, 0:128], wp[:, 128:256]
            vwt0, vwt1 = wp[:, 256:512], wp[:, 512:768]
            kw1, bmk = kb[:, 0 : 2 * MID], kb[:, 2 * MID :]
            u0 = sb.tile([128, HW], bf16, tag="u0", name="u0")
            u1 = sb.tile([128, HW], bf16, tag="u1", name="u1")
            for jo in range(0, 2048, 1024):
                nc.sync.dma_start(u0[:, jo : jo + 1024], u_in[0:128, jo : jo + 1024])
                nc.gpsimd.dma_start(u1[:, jo : jo + 1024],
                                    u_in[128:256, jo : jo + 1024])

            # ---- persistent SBUF state ----
            R2 = sb.tile([128, KH], bf16, tag="R2", name="R2")
            G2 = sb.tile([128, HW], bf16, tag="G2", name="G2")
            u_pairs = []
            for pair in range(NP):
                Up = sb.tile([128, 2 * HW], fp8, tag=f"Up{pair}", name=f"Up{pair}")
                u_pairs.append(Up)
            vtb = sb.tile([128, NK * C], bf16, tag="vtb", name="vtb")
            vtsp = []
            for pair in range(NP):
                vp = sb.tile([128, 2 * C], fp8, tag=f"vtsp{pair}", name=f"vtsp{pair}")
                vtsp.append(vp)
            s_v = sb.tile([128, NK], f32, tag="s_v", name="s_v")
            rinv = sb.tile([128, NK], f32, tag="rinv", name="rinv")
            sp_tail = {}
            for kt in (NK - 2, NK - 1):
                sp_tail[kt] = sb.tile([128, NCH], f32, tag=f"sp{kt}",
                                      name=f"sp{kt}")

            # stride-2 view of u: col = blk*256 + q*2 + two  (keys of pair blk)
            u0v = u0.rearrange("p (blk q two) -> p blk two q", two=2, q=128)
            u1v = u1.rearrange("p (blk q two) -> p blk two q", two=2, q=128)

            # ---- R2 = relu(kw1f2 outer bmk + beta2)  [128, 2048] ----
            pr = ps.tile([128, JC], f32, tag="pe", name="pr")
            for q in range(4):
                nc.tensor.matmul(
                    pr[:, q * 512 : (q + 1) * 512],
                    kw1[:, :], bmk[:, q * 512 : (q + 1) * 512],
                    start=True, stop=True,
                )
            # relu halves: chunk 0 gates the first energy tile, keep it first
            for rc in range(2):
                nc.vector.tensor_scalar(
                    R2[:, rc * 1024 : (rc + 1) * 1024],
                    pr[:, rc * 1024 : (rc + 1) * 1024],
                    betat[:, 0:1], 0.0, op0=ALU.add, op1=ALU.max,
                )

            # tail u chunks, emitted late so early consumers don't wait on them
            for jo in range(2048, HW, 1024):
                nc.sync.dma_start(u0[:, jo : jo + 1024], u_in[0:128, jo : jo + 1024])
                nc.gpsimd.dma_start(u1[:, jo : jo + 1024],
                                    u_in[128:256, jo : jo + 1024])

            # ---- G2 = M2 @ u (both halves)  [128, 4096] ----
            # chunk-0 cast on ACT (it idles pre-loop and gates the first exp),
            # chunk-1 cast on DVE in parallel.
            for ci in range(2):
                pg = ps.tile([128, JC], f32, tag="pe", name=f"pg{ci}")
                for q in range(4):
                    js = ci * JC + q * 512
                    sl = slice(q * 512, (q + 1) * 512)
                    nc.tensor.matmul(
                        pg[:, sl], mt[:, :], u0[:, js : js + 512],
                        start=True, stop=False,
                    )
                    nc.tensor.matmul(
                        pg[:, sl], mt1[:, :], u1[:, js : js + 512],
                        start=False, stop=True,
                    )
                if ci == 0:
                    nc.scalar.copy(G2[:, 0:JC], pg[:, :])
                else:
                    nc.vector.tensor_copy(G2[:, JC : 2 * JC], pg[:, :])

            def vt_round(r):
                """Vt for k-tiles 4r..4r+3, slipped into the buffer rotation."""
                pv = ps.tile([128, 1024], f32, tag="pe", name=f"pv{r}")
                for t in range(4):
                    kt = 4 * r + t
                    pair, hf = kt // 2, kt % 2
                    nc.tensor.matmul(
                        pv[:, t * 256 : (t + 1) * 256],
                        u0v[:, pair, hf, :], vwt0[:, :],
                        start=True, stop=False,
                    )
                    nc.tensor.matmul(
                        pv[:, t * 256 : (t + 1) * 256],
                        u1v[:, pair, hf, :], vwt1[:, :],
                        start=False, stop=True,
                    )
                nc.vector.tensor_copy(
                    vtb[:, 4 * r * 256 : (4 * r + 4) * 256], pv[:, :]
                )

            # ---- main loop: energy + exp + row-sums + scales ----
            for kt in range(NK):
                pair, hf = kt // 2, kt % 2
                Up = u_pairs[pair]
                ride = kt in sp_tail  # last pair: row-sum rides the ACT accum
                for ci in range(NCH):
                    pe = ps.tile([128, JC], f32, tag="pe", name=f"pe{kt}_{ci}")
                    js = ci * JC
                    for q in range(4):
                        h = q % 2
                        nc.tensor.matmul(
                            pe[:, q * 512 : (q + 1) * 512],
                            R2[h * 64 : (h + 1) * 64, kt * 128 : (kt + 1) * 128],
                            G2[h * 64 : (h + 1) * 64, js + q * 512 : js + (q + 1) * 512],
                            start=True, stop=True,
                        )
                    nc.scalar.activation(
                        Up[:, hf * HW + js : hf * HW + js + JC], pe[:, :], AF.Exp,
                        accum_out=(sp_tail[kt][:, ci : ci + 1] if ride else None),
                    )
                if ride:
                    nc.vector.reduce_sum(
                        s_v[:, kt : kt + 1], sp_tail[kt][:, :], axis=AX.X,
                    )
                else:
                    nc.vector.reduce_sum(
                        s_v[:, kt : kt + 1],
                        Up[:, hf * HW : hf * HW + HW], axis=AX.X,
                    )
                if kt < 4:
                    vt_round(kt)
                if hf == 1:
                    nc.vector.reciprocal(
                        rinv[:, kt - 1 : kt + 1], s_v[:, kt - 1 : kt + 1]
                    )
                    for h2 in (0, 1):
                        nc.gpsimd.tensor_scalar(
                            vtsp[pair][:, h2 * C : (h2 + 1) * C],
                            vtb[:, (kt - 1 + h2) * C : (kt + h2) * C],
                            rinv[:, kt - 1 + h2 : kt + h2], VSCALE,
                            op0=ALU.mult, op1=ALU.mult,
                        )

            # ---- phase C: P = Vt^T @ U (fp8 DoubleRow), 4 groups of
            # [128,2048].  The first two groups pre-stream pairs 0..6 while
            # the final sum/scale chain completes; only pair-7 matmuls gate
            # on it.  Copies on the now-idle ACT engine. ----
            def pc_matmuls(po, jb, ct, pairs):
                for pair in pairs:
                    lhs = vtsp[pair].rearrange("p (i c) -> p i c", i=2)[
                        :, :, ct * 128 : (ct + 1) * 128
                    ]
                    rhs = u_pairs[pair].rearrange("p (i j) -> p i j", i=2)
                    for q in range(4):
                        js = jb * JC + q * 512
                        nc.tensor.matmul(
                            po[:, q * 512 : (q + 1) * 512],
                            lhs, rhs[:, :, js : js + 512],
                            start=(pair == 0), stop=(pair == NP - 1),
                            perf_mode=DR,
                        )

            def pc_writeout(po, jb, ct, split=False):
                ost = osp.tile([128, JC], bf16, tag="ost", name=f"ost{jb}_{ct}")
                q = nc.scalar if ct == 0 else nc.sync
                if split:
                    h = JC // 2
                    nc.scalar.copy(ost[:, 0:h], po[:, 0:h])
                    q.dma_start(
                        out_d[ct * 128 : (ct + 1) * 128,
                              jb * JC : jb * JC + h], ost[:, 0:h])
                    nc.scalar.copy(ost[:, h:JC], po[:, h:JC])
                    nc.scalar.dma_start(
                        out_d[ct * 128 : (ct + 1) * 128,
                              jb * JC + h : (jb + 1) * JC], ost[:, h:JC])
                else:
                    nc.scalar.copy(ost[:, :], po[:, :])
                    q.dma_start(
                        out_d[ct * 128 : (ct + 1) * 128,
                              jb * JC : (jb + 1) * JC], ost[:, :])

            po00 = ps.tile([128, JC], f32, tag="pe", name="po00")
            po01 = ps.tile([128, JC], f32, tag="pe", name="po01")
            pc_matmuls(po00, 0, 0, range(NP - 1))
            pc_matmuls(po01, 0, 1, range(NP - 1))
            pc_matmuls(po00, 0, 0, [NP - 1])
            pc_matmuls(po01, 0, 1, [NP - 1])
            pc_writeout(po00, 0, 0)
            pc_writeout(po01, 0, 1)
            po10 = ps.tile([128, JC], f32, tag="pe", name="po10")
            pc_matmuls(po10, 1, 0, range(NP))
            pc_writeout(po10, 1, 0)
            po11 = ps.tile([128, JC], f32, tag="pe", name="po11")
            pc_matmuls(po11, 1, 1, range(NP))
            pc_writeout(po11, 1, 1, split=True)

    nc.compile()
    return nc


def _get_built():
    global _BUILT
    if _BUILT is None:
        _BUILT = _build()
    return _BUILT


def _kperm():
    """Pairwise interleave within 256-key blocks: new index kt*128+q maps to
    old key  (kt//2)*256 + 2q + (kt%2)."""
    perm = np.empty(KH, np.int64)
    for pair in range(NP):
        base = pair * 256
        perm[pair * 256 : pair * 256 + 128] = base + np.arange(0, 256, 2)
        perm[pair * 256 + 128 : pair * 256 + 256] = base + np.arange(1, 256, 2)
    return perm


def _host_prep(boundary_map, uncertainty_map, key_w1, bn_scale, bn_bias,
               bn_mean, bn_var, key_w2, query_w, value_w):
    import ml_dtypes

    bf16 = ml_dtypes.bfloat16
    b, c, h, w = uncertainty_map.shape
    H0 = boundary_map.shape[2]
    idx = (np.arange(h) * H0) // h
    bm = boundary_map[:, 0][:, idx][:, :, idx].reshape(b, h * w).astype(np.float32)

    inv = bn_scale / np.sqrt(bn_var + 1e-5)
    beta = (bn_bias - bn_mean * inv).astype(np.float32)
    kw1f = (key_w1[:, 0] * inv).astype(np.float32)
    m_t = np.ascontiguousarray((key_w2.T @ query_w).T).astype(np.float32)  # [256, 64]
    # duplicate across partition halves for the energy duo-packing
    kw1f2 = np.concatenate([kw1f, kw1f]).reshape(1, 2 * MID).astype(bf16)
    beta2 = np.concatenate([beta, beta]).reshape(2 * MID, 1).astype(np.float32)
    m_t2 = np.concatenate([m_t, m_t], axis=1).astype(bf16)                 # [256, 128]
    vw_t = value_w.T.astype(bf16)                                          # [256, 256]
    # packed weights: [mt | mt1 | vwt0 | vwt1]  [128, 768]
    wp = np.ascontiguousarray(np.concatenate(
        [m_t2[0:128], m_t2[128:256], vw_t[0:128], vw_t[128:256]], axis=1))
    perm = _kperm()

    in_maps = []
    for core in range(8):
        bi, kh = core // 2, core % 2
        u = uncertainty_map[bi].reshape(c, h * w).astype(np.float32)
        # roll j so this core's own key half occupies columns [0, KH)
        u_dev = np.roll(u, -KH * kh, axis=1).astype(bf16)
        bmk = bm[bi, kh * KH : (kh + 1) * KH][perm]
        kb = np.concatenate(
            [kw1f2, bmk.reshape(1, KH).astype(bf16)], axis=1)  # [1, 128+2048]
        in_maps.append({
            "u_in": np.ascontiguousarray(u_dev),
            "wp_in": wp,
            "kb_in": np.ascontiguousarray(kb),
            "beta_in": beta2,
        })
    return in_maps


def kernel(boundary_map, uncertainty_map, key_w1, bn_scale, bn_bias,
           bn_mean, bn_var, key_w2, query_w, value_w, gamma):
    global LAST_RESULTS
    from concourse.bass_utils import run_bass_kernel_spmd

    nc = _get_built()
    in_maps = _host_prep(
        np.asarray(boundary_map), np.asarray(uncertainty_map), np.asarray(key_w1),
        np.asarray(bn_scale), np.asarray(bn_bias), np.asarray(bn_mean),
        np.asarray(bn_var), np.asarray(key_w2), np.asarray(query_w),
        np.asarray(value_w),
    )
    kwargs = {}
    if TRACE:
        kwargs["trace"] = True
        if TRACE_CORES is not None:
            kwargs["trace_cores"] = TRACE_CORES
    res = run_bass_kernel_spmd(nc, in_maps, core_ids=list(range(8)), **kwargs)
    LAST_RESULTS = res

    b, c, h, w = uncertainty_map.shape
    g = np.float32(np.asarray(gamma).reshape(-1)[0] / VSCALE)
    out = np.empty((b, c, h * w), np.float32)
    um = np.asarray(uncertainty_map)
    for bi in range(b):
        P0 = res.results[2 * bi]["outp"].astype(np.float32)
        P1 = res.results[2 * bi + 1]["outp"].astype(np.float32)
        P = P0 + np.roll(P1, KH, axis=1)  # un-roll core kh=1's j columns
        out[bi] = g * P + um[bi].reshape(c, h * w)
    return out.reshape(b, c, h, w)
